# revision 1
# baseline (speedup 1.0000x reference)
"""Trainium2 Bass kernel for a dense transformer block (pre-LN, 16-head causal
attention + 3x FFN), distributed over 8 NeuronCores.

Sharding: tensor-parallel over heads (2 heads/core, both batch elements on
every core) for LN1/QKV/attention; one 8-core AllToAll redistributes the
per-head attention context to token-parallel shards (512 tokens/core) for the
output projection, LN2 and the FFN.  Matmuls run in bf16 with f32 PSUM
accumulation; the residual stream stays f32.

All layouts are transposed ([channel, token]) on chip so every matmul
contracts over the partition dim.  LayerNorm 1 is folded into the QKV weights:
q = inv_std[t] * (x @ Wq_eff - mu[t] * colsum(Wq_eff)) + be1 @ Wq, implemented
with a rank-2 correction matmul appended to each accumulation group.
"""

import numpy as np
import ml_dtypes

B, T, C = 2, 2048, 1024
NH, H = 16, 64
FF = 3 * C
EPS = 1e-6
N_CORES = 8
TT = B * T            # 4096 tokens processed per core (head-parallel phase)
TS = TT // N_CORES    # 512 tokens per core (token-parallel phase)
HPC = NH // N_CORES   # 2 heads per core
HD2 = HPC * H         # 128

BF16 = ml_dtypes.bfloat16

_BUILT = {}

NT = TT // 128        # 32 token tiles
NKC = C // 128        # 8 channel k-tiles
NMF = FF // 128       # 24 ff tiles


def _build():
    import concourse.bacc as bacc
    import concourse.mybir as mybir
    import concourse.tile as tile
    dt = mybir.dt
    alu = mybir.AluOpType
    act = mybir.ActivationFunctionType

    nc = bacc.Bacc("TRN2", target_bir_lowering=False, debug=False,
                   num_devices=N_CORES)

    # ----- kernel I/O (per-core shards) -----
    p_x = nc.declare_dram_parameter("p_x", [TT // N_CORES, C], dt.bfloat16, isOutput=False)
    p_xT = nc.declare_dram_parameter("p_xT", [C, TT], dt.bfloat16, isOutput=False)
    p_xTs = nc.declare_dram_parameter("p_xTs", [C, TS], dt.float32, isOutput=False)
    p_wq = nc.declare_dram_parameter("p_wq", [C, HD2], dt.bfloat16, isOutput=False)
    p_wk = nc.declare_dram_parameter("p_wk", [C, HD2], dt.bfloat16, isOutput=False)
    p_wv = nc.declare_dram_parameter("p_wv", [C, HD2], dt.bfloat16, isOutput=False)
    p_cq = nc.declare_dram_parameter("p_cq", [2, HD2], dt.bfloat16, isOutput=False)
    p_ck = nc.declare_dram_parameter("p_ck", [2, HD2], dt.bfloat16, isOutput=False)
    p_cv = nc.declare_dram_parameter("p_cv", [2, HD2], dt.bfloat16, isOutput=False)
    p_woblk = nc.declare_dram_parameter("p_woblk", [NKC, C, 128], dt.bfloat16, isOutput=False)
    p_bo = nc.declare_dram_parameter("p_bo", [1, C], dt.bfloat16, isOutput=False)
    p_w1blk = nc.declare_dram_parameter("p_w1blk", [NMF, C, 128], dt.bfloat16, isOutput=False)
    p_b1c = nc.declare_dram_parameter("p_b1c", [128, NMF], dt.float32, isOutput=False)
    p_w2blk = nc.declare_dram_parameter("p_w2blk", [NKC, FF, 128], dt.bfloat16, isOutput=False)
    p_b2 = nc.declare_dram_parameter("p_b2", [1, C], dt.bfloat16, isOutput=False)
    p_maskd = nc.declare_dram_parameter("p_maskd", [128, 128], dt.bfloat16, isOutput=False)
    p_ident = nc.declare_dram_parameter("p_ident", [128, 128], dt.bfloat16, isOutput=False)
    p_out = nc.declare_dram_parameter("p_out", [C, TS], dt.float32, isOutput=True)

    with tile.TileContext(nc, num_cores=N_CORES) as tc:
        with (
            tc.tile_pool(name="persist", bufs=1) as pp,
            tc.tile_pool(name="dram", bufs=1, space="DRAM") as pdram,
        ):
            # ------------- persistent constants & activation tensors -------------
            ident = pp.tile([128, 128], dt.bfloat16)
            nc.sync.dma_start(ident[:], p_ident[:])
            maskd = pp.tile([128, 128], dt.bfloat16)
            nc.sync.dma_start(maskd[:], p_maskd[:])
            ones_row = pp.tile([1, 512], dt.bfloat16)
            nc.vector.memset(ones_row[:], 1.0)
            ones128_row = pp.tile([1, 128], dt.bfloat16)
            nc.vector.memset(ones128_row[:], 1.0)
            isc_col = pp.tile([128, 1], dt.bfloat16)   # 1/1024 column for LN2 sums
            nc.vector.memset(isc_col[:], 1.0 / C)

            cq = pp.tile([2, HD2], dt.bfloat16)
            nc.sync.dma_start(cq[:], p_cq[:])
            ck = pp.tile([2, HD2], dt.bfloat16)
            nc.sync.dma_start(ck[:], p_ck[:])
            cv = pp.tile([2, HD2], dt.bfloat16)
            nc.sync.dma_start(cv[:], p_cv[:])

            # QKV weights: [C, 128] -> [128, 8, 128] (k-tile at [:, k, :])
            wq = pp.tile([128, NKC, HD2], dt.bfloat16)
            nc.sync.dma_start(wq[:], p_wq.ap().rearrange("(k p) h -> p k h", p=128))
            wk = pp.tile([128, NKC, HD2], dt.bfloat16)
            nc.sync.dma_start(wk[:], p_wk.ap().rearrange("(k p) h -> p k h", p=128))
            wv = pp.tile([128, NKC, HD2], dt.bfloat16)
            nc.sync.dma_start(wv[:], p_wv.ap().rearrange("(k p) h -> p k h", p=128))

            # rows_all [2, TT]: row 0 = -mu, row 1 = std+eps; inv_row [1, TT]
            rows_all = pp.tile([2, TT], dt.bfloat16)
            inv_row = pp.tile([1, TT], dt.bfloat16)
            inv_b = pp.tile([128, TT], dt.bfloat16)
            invf = pp.tile([128, NT], dt.float32)
            qT = pp.tile([128, TT], dt.bfloat16)
            kT = pp.tile([128, TT], dt.bfloat16)
            v = pp.tile([128, NT, 2, 65], dt.bfloat16)
            ctxT = pp.tile([128, TT], dt.bfloat16)

            # ---------------- stage A: LN1 stats (sharded) + QKV ----------------
            with (
                tc.tile_pool(name="xtpool", bufs=1) as pxt,
                tc.tile_pool(name="xin", bufs=4) as px,
                tc.tile_pool(name="stat", bufs=1) as pst,
                tc.tile_pool(name="apsum", bufs=3, space="PSUM") as pps_a,
                tc.tile_pool(name="apsum1", bufs=1, space="PSUM") as pps_a1,
            ):
                # local bn_stats over this core's 4 token tiles
                NLT = NT // N_CORES        # 4 local token tiles
                stats = pst.tile([128, NLT, 2], dt.float32)
                for i in range(NLT):
                    xt = px.tile([128, C], dt.bfloat16, tag="xtc")
                    nc.sync.dma_start(xt[:], p_x[128 * i:128 * (i + 1), :])
                    bnt = px.tile([128, 2, 6], dt.float32, tag="bnt")
                    nc.vector.bn_stats(bnt[:, 0, :], xt[:, 0:512])
                    nc.vector.bn_stats(bnt[:, 1, :], xt[:, 512:1024])
                    nc.vector.bn_aggr(stats[:, i, :], bnt[:])

                # (negmu, std+eps, inv) for the local 512 tokens
                stat2 = pst.tile([128, NLT, 2], dt.bfloat16)
                stdf = pst.tile([128, NLT], dt.float32)
                nc.scalar.activation(stdf[:], stats[:, :, 1], act.Sqrt,
                                     scale=float(C) / (C - 1))
                nc.vector.tensor_scalar(stdf[:], stdf[:], EPS, None, alu.add)
                invf = pst.tile([128, NLT], dt.float32)
                nc.vector.reciprocal(invf[:], stdf[:])
                nc.vector.tensor_scalar(stat2[:, :, 0], stats[:, :, 0], -1.0, None,
                                        alu.mult)
                nc.vector.tensor_copy(stat2[:, :, 1], stdf[:])
                statinv = pst.tile([128, NLT], dt.bfloat16)
                nc.vector.tensor_copy(statinv[:], invf[:])

                # local rows: (negmu, std+eps) [2, 512] and inv [1, 512]
                rows_loc = pst.tile([2, TS], dt.bfloat16)
                rows_locv = pst.tile([1, TS], dt.bfloat16)
                for i in range(NLT):
                    pt = pps_a1.tile([2, 128], dt.bfloat16, tag="rowtp")
                    nc.tensor.transpose(pt[:], stat2[:, i, :], ident[:])
                    nc.vector.tensor_copy(rows_loc[:, 128 * i:128 * (i + 1)], pt[:])
                    ptv = pps_a1.tile([1, 128], dt.bfloat16, tag="rowtpv")
                    nc.tensor.transpose(ptv[:], statinv[:, i:i + 1], ident[:])
                    nc.vector.tensor_copy(rows_locv[:, 128 * i:128 * (i + 1)], ptv[:])

                # all-gather the stat rows (tiny, latency-bound)
                st_in = pdram.tile([3, TS], dt.bfloat16)
                st_out = pdram.tile([N_CORES, 3, TS], dt.bfloat16)
                nc.sync.dma_start(st_in[0:2, :], rows_loc[:])
                nc.sync.dma_start(st_in[2:3, :], rows_locv[:])
                nc.gpsimd.collective_compute(
                    "AllGather", alu.bypass,
                    replica_groups=[list(range(N_CORES))],
                    ins=[st_in.opt()],
                    outs=[st_out.opt()],
                )
                for r in range(N_CORES):
                    nc.sync.dma_start(rows_all[:, TS * r:TS * (r + 1)], st_out[r, 0:2, :])
                    nc.sync.dma_start(inv_row[:, TS * r:TS * (r + 1)], st_out[r, 2:3, :])

                # inv broadcast down partitions (evict on scalar engine)
                for ch in range(TT // 512):
                    pb = pps_a1.tile([128, 512], dt.float32, tag="invb")
                    nc.tensor.matmul(pb[:], ones128_row[:],
                                     inv_row[0:1, 512 * ch:512 * (ch + 1)],
                                     start=True, stop=True)
                    nc.scalar.copy(inv_b[:, 512 * ch:512 * (ch + 1)], pb[:])

                # x^T resident for the QKV matmuls, DMA'd per token-chunk
                xT = pxt.tile([128, NKC, TT], dt.bfloat16)
                for ch in range(TT // 512):
                    nc.sync.dma_start(
                        xT[:, :, 512 * ch:512 * (ch + 1)],
                        p_xT.ap()[:, 512 * ch:512 * (ch + 1)].rearrange(
                            "(k p) t -> p k t", p=128))

                vT = pxt.tile([128, TT], dt.bfloat16)
                for ch in range(TT // 512):
                    sl = slice(512 * ch, 512 * (ch + 1))
                    for (nm, w, cw, dst) in (("q", wq, cq, qT), ("k", wk, ck, kT),
                                             ("v", wv, cv, vT)):
                        ps = pps_a.tile([128, 512], dt.float32,
                                        name=f"ps{nm}", tag="qkv")
                        for k in range(NKC):
                            nc.tensor.matmul(ps[:], w[:, k, :], xT[:, k, sl],
                                             start=(k == 0), stop=False)
                        nc.tensor.matmul(ps[:], cw[:], rows_all[0:2, sl],
                                         start=False, stop=True)
                        nc.vector.tensor_tensor(dst[:, sl], ps[:], inv_b[:, sl],
                                                alu.mult)

                # v_aug [s, tile, head, 65] via PE transpose of vT; col 64 = 1
                nc.vector.memset(v[:, :, :, 64], 1.0)
                for i in range(NT):
                    pvt = pps_a1.tile([128, 128], dt.bfloat16, tag="vtp")
                    nc.tensor.transpose(pvt[:], vT[:, 128 * i:128 * (i + 1)],
                                        ident[:])
                    nc.scalar.copy(v[:, i, :, 0:64],
                                   pvt[:].rearrange("p (h d) -> p h d", h=2))

            # ---------------- stage B: attention ----------------
            with (
                tc.tile_pool(name="exps", bufs=6) as pexp,
                tc.tile_pool(name="attsb", bufs=2) as pat,
                tc.tile_pool(name="scpsum", bufs=3, space="PSUM") as pps_sc,
                tc.tile_pool(name="ctxpsum", bufs=2, space="PSUM") as pps_ctx,
                tc.tile_pool(name="zbpsum", bufs=1, space="PSUM") as pps_zb,
            ):
                for b in range(B):
                    for qt in range(T // 512):
                        G = b * T + 512 * qt
                        gsl = slice(G, G + 512)
                        nj = 4 * qt + 4
                        pc = [pps_ctx.tile([65, 512], dt.float32,
                                           name=f"pc{h}", tag=f"ctx{h}")
                              for h in range(2)]
                        ets = []
                        for j in range(nj):
                            st = b * (T // 128) + j   # global s-tile index
                            et2 = []
                            for h in range(2):
                                hsl = slice(64 * h, 64 * (h + 1))
                                ps = pps_sc.tile([128, 512], dt.float32,
                                                 name=f"ps{h}", tag="sc")
                                nc.tensor.matmul(
                                    ps[:], kT[hsl, 128 * st:128 * (st + 1)],
                                    qT[hsl, gsl], start=True, stop=True)
                                et = pexp.tile([128, 512], dt.bfloat16,
                                               name=f"et{h}", tag=f"et{h}")
                                if j >= nj - 4:
                                    off = j - (nj - 4)
                                    if off > 0:
                                        nc.gpsimd.memset(et[:, 0:128 * off], 0.0)
                                    nc.scalar.activation(
                                        et[:, 128 * off:512], ps[:, 128 * off:512],
                                        act.Exp, scale=1.0 / float(np.sqrt(H)))
                                    nc.gpsimd.tensor_tensor(
                                        et[:, 128 * off:128 * (off + 1)],
                                        et[:, 128 * off:128 * (off + 1)],
                                        maskd[:], alu.mult)
                                else:
                                    nc.scalar.activation(et[:], ps[:], act.Exp,
                                                         scale=1.0 / float(np.sqrt(H)))
                                et2.append(et)
                            ets.append(et2)
                            # software pipeline: AV for tile j-1 after scores of j
                            if j > 0:
                                for h in range(2):
                                    nc.tensor.matmul(
                                        pc[h][:], v[:, b * (T // 128) + j - 1, h, :],
                                        ets[j - 1][h][:],
                                        start=(j - 1 == 0), stop=False)
                        for h in range(2):
                            nc.tensor.matmul(
                                pc[h][:], v[:, b * (T // 128) + nj - 1, h, :],
                                ets[nj - 1][h][:],
                                start=(nj == 1), stop=True)
                        # normalize by Z (row 64 of each ctx psum)
                        pzb = pps_zb.tile([128, 512], dt.float32, tag="zb")
                        for h in range(2):
                            zrow = pat.tile([1, 512], dt.float32,
                                            name=f"zrow{h}", tag=f"z{h}")
                            nc.vector.tensor_copy(zrow[:], pc[h][64:65, :])
                            zinv = pat.tile([1, 512], dt.float32,
                                            name=f"zinv{h}", tag=f"zi{h}")
                            nc.vector.reciprocal(zinv[:], zrow[:])
                            zinvb = pat.tile([1, 512], dt.bfloat16,
                                             name=f"zinvb{h}", tag=f"zib{h}")
                            nc.vector.tensor_copy(zinvb[:], zinv[:])
                            nc.tensor.matmul(pzb[64 * h:64 * (h + 1), :],
                                             ones128_row[0:1, 0:64], zinvb[:],
                                             start=True, stop=True)
                        zb = pat.tile([128, 512], dt.bfloat16, tag="zbs")
                        nc.vector.tensor_copy(zb[:], pzb[:])
                        for h in range(2):
                            nc.vector.tensor_tensor(
                                ctxT[64 * h:64 * (h + 1), gsl],
                                pc[h][0:64, :], zb[64 * h:64 * (h + 1), :],
                                alu.mult)

            # ---------------- AllToAll: heads -> tokens ----------------
            cc_in = pdram.tile([N_CORES, 128, TS], dt.bfloat16)
            cc_out = pdram.tile([N_CORES, 128, TS], dt.bfloat16)
            for j in range(N_CORES):
                nc.sync.dma_start(cc_in[j], ctxT[:, TS * j:TS * (j + 1)])
            nc.gpsimd.collective_compute(
                "AllToAll", alu.bypass,
                replica_groups=[list(range(N_CORES))],
                ins=[cc_in.opt()],
                outs=[cc_out.opt()],
            )

            # ---------------- stage C: Wo + LN2 + FFN ----------------
            with (
                tc.tile_pool(name="postsb", bufs=1) as pq,
                tc.tile_pool(name="wstream", bufs=2) as pw,
                tc.tile_pool(name="evict", bufs=3) as pev,
                tc.tile_pool(name="ln2tmp", bufs=1) as pl2,
                tc.tile_pool(name="ffpsum", bufs=2, space="PSUM") as pps_ff,
                tc.tile_pool(name="cpsum", bufs=1, space="PSUM") as pps_c,
            ):
                ctxF = pq.tile([128, NKC, TS], dt.bfloat16)
                for j in range(N_CORES):
                    nc.sync.dma_start(ctxF[:, j, :], cc_out[j])

                bo = pq.tile([1, C], dt.bfloat16)
                nc.sync.dma_start(bo[:], p_bo[:])
                b2 = pq.tile([1, C], dt.bfloat16)
                nc.sync.dma_start(b2[:], p_b2[:])
                b1c = pq.tile([128, NMF], dt.float32)
                nc.sync.dma_start(b1c[:], p_b1c[:])
                xTs = pq.tile([128, NKC, TS], dt.float32)
                nc.sync.dma_start(xTs[:], p_xTs.ap().rearrange("(k p) t -> p k t", p=128))

                r2T = pq.tile([128, NKC, TS], dt.float32)
                for mc in range(NKC):
                    wo_blk = pw.tile([128, NKC, 128], dt.bfloat16, tag="wo")
                    nc.sync.dma_start(
                        wo_blk[:],
                        p_woblk[mc].rearrange("(k p) c -> p k c", p=128))
                    ps = pps_ff.tile([128, TS], dt.float32, tag="ff")
                    for k in range(NKC):
                        nc.tensor.matmul(ps[:], wo_blk[:, k, :], ctxF[:, k, :],
                                         start=(k == 0), stop=False)
                    nc.tensor.matmul(ps[:], bo[0:1, 128 * mc:128 * (mc + 1)],
                                     ones_row[:], start=False, stop=True)
                    nc.vector.tensor_tensor(r2T[:, mc, :], ps[:], xTs[:, mc, :],
                                            alu.add)

                # ---- LN2 over the channel (partition) dim via PE sums ----
                r2b = pl2.tile([128, NKC, TS], dt.bfloat16)
                sq = pl2.tile([128, NKC, TS], dt.bfloat16)
                for mc in range(NKC):
                    nc.scalar.copy(r2b[:, mc, :], r2T[:, mc, :])
                    nc.vector.tensor_tensor(sq[:, mc, :], r2b[:, mc, :],
                                            r2b[:, mc, :], alu.mult)
                ps1 = pps_c.tile([1, TS], dt.float32, tag="s1")
                ps2 = pps_c.tile([1, TS], dt.float32, tag="s2")
                for mc in range(NKC):
                    nc.tensor.matmul(ps1[:], isc_col[:], r2b[:, mc, :],
                                     start=(mc == 0), stop=(mc == NKC - 1))
                for mc in range(NKC):
                    nc.tensor.matmul(ps2[:], isc_col[:], sq[:, mc, :],
                                     start=(mc == 0), stop=(mc == NKC - 1))
                muf = pl2.tile([1, TS], dt.float32)
                nc.vector.tensor_copy(muf[:], ps1[:])
                varf = pl2.tile([1, TS], dt.float32)
                nc.vector.tensor_tensor(varf[:], muf[:], muf[:], alu.mult)
                nc.vector.tensor_tensor(varf[:], ps2[:], varf[:], alu.subtract)
                stdf2 = pl2.tile([1, TS], dt.float32)
                nc.scalar.activation(stdf2[:], varf[:], act.Sqrt,
                                     scale=float(C) / (C - 1))
                nc.vector.tensor_scalar(stdf2[:], stdf2[:], EPS, None, alu.add)
                inv2 = pl2.tile([1, TS], dt.float32)
                nc.vector.reciprocal(inv2[:], stdf2[:])
                mu2row = pl2.tile([1, TS], dt.bfloat16)
                nc.vector.tensor_copy(mu2row[:], muf[:])
                inv2row = pl2.tile([1, TS], dt.bfloat16)
                nc.vector.tensor_copy(inv2row[:], inv2[:])
                pmb = pps_c.tile([128, TS], dt.float32, tag="bcast")
                nc.tensor.matmul(pmb[:], ones128_row[:], mu2row[:],
                                 start=True, stop=True)
                m2b = pl2.tile([128, TS], dt.bfloat16)
                nc.scalar.copy(m2b[:], pmb[:])
                pib = pps_c.tile([128, TS], dt.float32, tag="bcast")
                nc.tensor.matmul(pib[:], ones128_row[:], inv2row[:],
                                 start=True, stop=True)
                i2b = pl2.tile([128, TS], dt.bfloat16)
                nc.scalar.copy(i2b[:], pib[:])

                xn2T = pq.tile([128, NKC, TS], dt.bfloat16)
                for mc in range(NKC):
                    tmp = pev.tile([128, TS], dt.bfloat16, tag="xtmp")
                    nc.vector.tensor_tensor(tmp[:], r2T[:, mc, :], m2b[:],
                                            alu.subtract)
                    nc.vector.tensor_tensor(xn2T[:, mc, :], tmp[:], i2b[:],
                                            alu.mult)

                # ---- FFN ----
                hT = pq.tile([128, NMF, TS], dt.bfloat16)
                for mf in range(NMF):
                    w1_blk = pw.tile([128, NKC, 128], dt.bfloat16, tag="w1")
                    nc.sync.dma_start(
                        w1_blk[:],
                        p_w1blk[mf].rearrange("(k p) f -> p k f", p=128))
                    ps = pps_ff.tile([128, TS], dt.float32, tag="ff")
                    for k in range(NKC):
                        nc.tensor.matmul(ps[:], w1_blk[:, k, :], xn2T[:, k, :],
                                         start=(k == 0), stop=(k == NKC - 1))
                    nc.vector.tensor_scalar(hT[:, mf, :], ps[:], b1c[:, mf:mf + 1],
                                            0.0, alu.add, alu.max)

                for mc in range(NKC):
                    w2_blk = pw.tile([128, NMF, 128], dt.bfloat16, tag="w2")
                    nc.sync.dma_start(
                        w2_blk[:],
                        p_w2blk[mc].rearrange("(k p) c -> p k c", p=128))
                    ps = pps_ff.tile([128, TS], dt.float32, tag="ff")
                    for k in range(NMF):
                        nc.tensor.matmul(ps[:], w2_blk[:, k, :], hT[:, k, :],
                                         start=(k == 0), stop=False)
                    nc.tensor.matmul(ps[:], b2[0:1, 128 * mc:128 * (mc + 1)],
                                     ones_row[:], start=False, stop=True)
                    ot = pev.tile([128, TS], dt.float32, tag="ot")
                    nc.vector.tensor_tensor(ot[:], ps[:], r2T[:, mc, :], alu.add)
                    nc.sync.dma_start(p_out[128 * mc:128 * (mc + 1), :], ot[:])

    nc.compile()
    return nc


def _host_prep(inputs):
    """Fold layernorm affine params into weights; build per-core input maps."""
    x = np.asarray(inputs["x"], np.float32)
    Wq = np.asarray(inputs["Wq"], np.float32)
    Wk = np.asarray(inputs["Wk"], np.float32)
    Wv = np.asarray(inputs["Wv"], np.float32)
    Wo = np.asarray(inputs["Wo"], np.float32)
    bo = np.asarray(inputs["bo"], np.float32)
    W1 = np.asarray(inputs["W1"], np.float32)
    b1 = np.asarray(inputs["b1"], np.float32)
    W2 = np.asarray(inputs["W2"], np.float32)
    b2 = np.asarray(inputs["b2"], np.float32)
    g1 = np.asarray(inputs["g1"], np.float32)
    be1 = np.asarray(inputs["be1"], np.float32)
    g2 = np.asarray(inputs["g2"], np.float32)
    be2 = np.asarray(inputs["be2"], np.float32)

    xf = x.reshape(TT, C)                      # both batches stacked
    xT = np.ascontiguousarray(xf.T)            # [C, TT]

    def fold_qkv(W):
        Weff = g1[:, None] * W                  # [NH, C, H] with g1 on C
        Weff = np.ascontiguousarray(np.transpose(Weff, (1, 0, 2)))  # [C, NH, H]
        bias = np.einsum("c,hck->hk", be1, W)   # [NH, H]
        colsum = Weff.sum(axis=0)               # [NH, H]
        return Weff, bias, colsum

    Wq_e, bq, csq = fold_qkv(Wq)
    Wk_e, bk, csk = fold_qkv(Wk)
    Wv_e, bv, csv = fold_qkv(Wv)

    woT = np.ascontiguousarray(Wo.T)            # [NH*H, C]
    w1T = np.ascontiguousarray(g2[:, None] * W1.T)   # [C, FF]
    b1_eff = b1 + be2 @ W1.T                         # [FF]
    w2T = np.ascontiguousarray(W2.T)            # [FF, C]

    # blocked weights: [nblocks, K, 128] with contiguous [K, 128] blocks
    woblk = np.ascontiguousarray(
        woT.reshape(C, NKC, 128).transpose(1, 0, 2))
    w1blk = np.ascontiguousarray(
        w1T.reshape(C, NMF, 128).transpose(1, 0, 2))
    w2blk = np.ascontiguousarray(
        w2T.reshape(FF, NKC, 128).transpose(1, 0, 2))

    tq = np.arange(128)[None, :]
    s = np.arange(128)[:, None]
    maskd = (s <= tq).astype(BF16)

    x_bf = xf.astype(BF16)
    shared = {
        "p_xT": xT.astype(BF16),
        "p_woblk": woblk.astype(BF16),
        "p_bo": bo[None, :].astype(BF16),
        "p_w1blk": w1blk.astype(BF16),
        "p_b1c": np.ascontiguousarray(
            b1_eff.reshape(NMF, 128).T).astype(np.float32),
        "p_w2blk": w2blk.astype(BF16),
        "p_b2": b2[None, :].astype(BF16),
        "p_maskd": maskd,
        "p_ident": np.eye(128, dtype=np.float32).astype(BF16),
    }

    in_maps = []
    for r in range(N_CORES):
        h0 = HPC * r
        hs = slice(h0, h0 + HPC)
        b_r, s_r = divmod(r, N_CORES // B)
        tok = slice(s_r * TS, (s_r + 1) * TS)
        xTs = np.ascontiguousarray(x[b_r].T[:, tok])
        m = dict(shared)
        m["p_x"] = x_bf[r * (TT // N_CORES):(r + 1) * (TT // N_CORES), :]
        m["p_xTs"] = xTs.astype(np.float32)
        m["p_wq"] = np.ascontiguousarray(
            Wq_e[:, hs, :].reshape(C, HD2)).astype(BF16)
        m["p_wk"] = np.ascontiguousarray(
            Wk_e[:, hs, :].reshape(C, HD2)).astype(BF16)
        m["p_wv"] = np.ascontiguousarray(
            Wv_e[:, hs, :].reshape(C, HD2)).astype(BF16)
        m["p_cq"] = np.stack([csq[hs].reshape(HD2),
                              bq[hs].reshape(HD2)]).astype(BF16)
        m["p_ck"] = np.stack([csk[hs].reshape(HD2),
                              bk[hs].reshape(HD2)]).astype(BF16)
        m["p_cv"] = np.stack([csv[hs].reshape(HD2),
                              bv[hs].reshape(HD2)]).astype(BF16)
        in_maps.append(m)
    return in_maps


def kernel(**inputs) -> np.ndarray:
    from concourse.bass_utils import run_bass_kernel_spmd

    if "nc" not in _BUILT:
        _BUILT["nc"] = _build()
    nc = _BUILT["nc"]

    in_maps = _host_prep(inputs)
    res = run_bass_kernel_spmd(nc, in_maps, core_ids=list(range(N_CORES)))

    out = np.empty((B, T, C), np.float32)
    for r in range(N_CORES):
        b_r, s_r = divmod(r, N_CORES // B)
        out[b_r, s_r * TS:(s_r + 1) * TS, :] = res.results[r]["p_out"].T
    return out



# revision 4
# speedup vs baseline: 1.1202x; 1.1202x over previous
"""Trainium2 Bass kernel for a dense transformer block (pre-LN, 16-head causal
attention + 3x FFN), distributed over 8 NeuronCores.

Sharding: tensor-parallel over heads (2 heads/core, both batch elements on
every core) for LN1/QKV/attention; one 8-core AllToAll redistributes the
per-head attention context to token-parallel shards (512 tokens/core) for the
output projection, LN2 and the FFN.  Matmuls run in bf16 with f32 PSUM
accumulation; the residual stream stays f32.

All layouts are transposed ([channel, token]) on chip so every matmul
contracts over the partition dim.  LayerNorm 1 is folded into the QKV weights:
q = inv_std[t] * (x @ Wq_eff - mu[t] * colsum(Wq_eff)) + be1 @ Wq, implemented
with a rank-2 correction matmul appended to each accumulation group.

v2 perf changes vs baseline:
 - all DRAM tensors laid out so DMAs are >=2KB contiguous per partition
 - softmax 1/Z via reciprocal_approx_fast; LN inverses via Rsqrt
 - softmax normalization software-pipelined across q-chunks (PE never
   queues behind the Z chain)
 - AllToAll input DMAs issued per-chunk inside the attention loop
 - LN2 column-sum matmuls interleaved into the Wo loop
 - deeper FFN weight prefetch (bufs=3)
"""

import numpy as np
import ml_dtypes

B, T, C = 2, 2048, 1024
NH, H = 16, 64
FF = 3 * C
EPS = 1e-6
N_CORES = 8
TT = B * T            # 4096 tokens processed per core (head-parallel phase)
TS = TT // N_CORES    # 512 tokens per core (token-parallel phase)
HPC = NH // N_CORES   # 2 heads per core
HD2 = HPC * H         # 128

BF16 = ml_dtypes.bfloat16

_BUILT = {}

NT = TT // 128        # 32 token tiles
NKC = C // 128        # 8 channel k-tiles
NMF = FF // 128       # 24 ff tiles
NCH = TT // 512       # 8 512-token chunks


def _build():
    import concourse.bacc as bacc
    import concourse.mybir as mybir
    import concourse.tile as tile
    dt = mybir.dt
    alu = mybir.AluOpType
    act = mybir.ActivationFunctionType

    nc = bacc.Bacc("TRN2", target_bir_lowering=False, debug=False,
                   num_devices=N_CORES)

    # ----- kernel I/O (per-core shards) -----
    # layouts chosen so every DMA is contiguous per SBUF partition
    p_x = nc.declare_dram_parameter("p_x", [TT // N_CORES, C], dt.bfloat16, isOutput=False)
    p_xT = nc.declare_dram_parameter("p_xT", [NCH, 128, NKC, 512], dt.bfloat16, isOutput=False)
    p_xTs = nc.declare_dram_parameter("p_xTs", [128, NKC, TS], dt.float32, isOutput=False)
    p_wq = nc.declare_dram_parameter("p_wq", [128, NKC, HD2], dt.bfloat16, isOutput=False)
    p_wk = nc.declare_dram_parameter("p_wk", [128, NKC, HD2], dt.bfloat16, isOutput=False)
    p_wv = nc.declare_dram_parameter("p_wv", [128, NKC, HD2], dt.bfloat16, isOutput=False)
    p_cq = nc.declare_dram_parameter("p_cq", [2, HD2], dt.bfloat16, isOutput=False)
    p_ck = nc.declare_dram_parameter("p_ck", [2, HD2], dt.bfloat16, isOutput=False)
    p_cv = nc.declare_dram_parameter("p_cv", [2, HD2], dt.bfloat16, isOutput=False)
    p_woblk = nc.declare_dram_parameter("p_woblk", [NKC, 128, NKC, 128], dt.bfloat16, isOutput=False)
    p_bo = nc.declare_dram_parameter("p_bo", [1, C], dt.bfloat16, isOutput=False)
    p_w1blk = nc.declare_dram_parameter("p_w1blk", [NMF, 128, NKC, 128], dt.bfloat16, isOutput=False)
    p_b1c = nc.declare_dram_parameter("p_b1c", [128, NMF], dt.float32, isOutput=False)
    p_w2blk = nc.declare_dram_parameter("p_w2blk", [NKC, 128, NMF, 128], dt.bfloat16, isOutput=False)
    p_b2 = nc.declare_dram_parameter("p_b2", [1, C], dt.bfloat16, isOutput=False)
    p_maskd = nc.declare_dram_parameter("p_maskd", [128, 128], dt.bfloat16, isOutput=False)
    p_ident = nc.declare_dram_parameter("p_ident", [128, 128], dt.bfloat16, isOutput=False)
    p_out = nc.declare_dram_parameter("p_out", [C, TS], dt.float32, isOutput=True)

    with tile.TileContext(nc, num_cores=N_CORES) as tc:
        with (
            tc.tile_pool(name="persist", bufs=1) as pp,
            tc.tile_pool(name="dram", bufs=1, space="DRAM") as pdram,
        ):
            # ------------- persistent constants & activation tensors -------------
            ident = pp.tile([128, 128], dt.bfloat16)
            nc.sync.dma_start(ident[:], p_ident[:])
            maskd = pp.tile([128, 128], dt.bfloat16)
            nc.sync.dma_start(maskd[:], p_maskd[:])
            ones_row = pp.tile([1, 512], dt.bfloat16)
            nc.vector.memset(ones_row[:], 1.0)
            ones128_row = pp.tile([1, 128], dt.bfloat16)
            nc.vector.memset(ones128_row[:], 1.0)
            isc_col = pp.tile([128, 1], dt.bfloat16)   # 1/1024 column for LN2 sums
            nc.vector.memset(isc_col[:], 1.0 / C)

            cq = pp.tile([2, HD2], dt.bfloat16)
            nc.sync.dma_start(cq[:], p_cq[:])
            ck = pp.tile([2, HD2], dt.bfloat16)
            nc.sync.dma_start(ck[:], p_ck[:])
            cv = pp.tile([2, HD2], dt.bfloat16)
            nc.sync.dma_start(cv[:], p_cv[:])

            # QKV weights [128, k, hd2], contiguous per partition in DRAM
            wq = pp.tile([128, NKC, HD2], dt.bfloat16)
            nc.sync.dma_start(wq[:], p_wq[:])
            wk = pp.tile([128, NKC, HD2], dt.bfloat16)
            nc.sync.dma_start(wk[:], p_wk[:])
            wv = pp.tile([128, NKC, HD2], dt.bfloat16)
            nc.sync.dma_start(wv[:], p_wv[:])

            # rows_all [2, TT]: row 0 = -mu, row 1 = std+eps; inv_row [1, TT]
            rows_all = pp.tile([2, TT], dt.bfloat16)
            inv_row = pp.tile([1, TT], dt.bfloat16)
            inv_b = pp.tile([128, TT], dt.bfloat16)
            qT = pp.tile([128, TT], dt.bfloat16)
            kT = pp.tile([128, TT], dt.bfloat16)
            v = pp.tile([128, NT, 2, 65], dt.bfloat16)
            ctxT = pp.tile([128, TT], dt.bfloat16)

            cc_in = pdram.tile([N_CORES, 128, TS], dt.bfloat16)
            cc_out = pdram.tile([N_CORES, 128, TS], dt.bfloat16)

            # ---------------- stage A: LN1 stats (sharded) + QKV ----------------
            with (
                tc.tile_pool(name="xtpool", bufs=1) as pxt,
                tc.tile_pool(name="xin", bufs=4) as px,
                tc.tile_pool(name="stat", bufs=1) as pst,
                tc.tile_pool(name="apsum", bufs=3, space="PSUM") as pps_a,
                tc.tile_pool(name="apsum1", bufs=1, space="PSUM") as pps_a1,
            ):
                # local bn_stats over this core's 4 token tiles
                NLT = NT // N_CORES        # 4 local token tiles
                stats = pst.tile([128, NLT, 2], dt.float32)
                for i in range(NLT):
                    xt = px.tile([128, C], dt.bfloat16, tag="xtc")
                    nc.sync.dma_start(xt[:], p_x[128 * i:128 * (i + 1), :])
                    bnt = px.tile([128, 2, 6], dt.float32, tag="bnt")
                    nc.vector.bn_stats(bnt[:, 0, :], xt[:, 0:512])
                    nc.vector.bn_stats(bnt[:, 1, :], xt[:, 512:1024])
                    nc.vector.bn_aggr(stats[:, i, :], bnt[:])

                # (negmu, std+eps, inv) for the local 512 tokens
                stat2 = pst.tile([128, NLT, 2], dt.bfloat16)
                stdf = pst.tile([128, NLT], dt.float32)
                nc.scalar.activation(stdf[:], stats[:, :, 1], act.Sqrt,
                                     scale=float(C) / (C - 1))
                nc.vector.tensor_scalar(stdf[:], stdf[:], EPS, None, alu.add)
                invf = pst.tile([128, NLT], dt.float32)
                nc.vector.reciprocal(invf[:], stdf[:])
                nc.vector.tensor_scalar(stat2[:, :, 0], stats[:, :, 0], -1.0, None,
                                        alu.mult)
                nc.vector.tensor_copy(stat2[:, :, 1], stdf[:])
                statinv = pst.tile([128, NLT], dt.bfloat16)
                nc.vector.tensor_copy(statinv[:], invf[:])

                # local rows: (negmu, std+eps) [2, 512] and inv [1, 512]
                rows_loc = pst.tile([2, TS], dt.bfloat16)
                rows_locv = pst.tile([1, TS], dt.bfloat16)
                for i in range(NLT):
                    pt = pps_a1.tile([2, 128], dt.bfloat16, tag="rowtp")
                    nc.tensor.transpose(pt[:], stat2[:, i, :], ident[:])
                    nc.vector.tensor_copy(rows_loc[:, 128 * i:128 * (i + 1)], pt[:])
                    ptv = pps_a1.tile([1, 128], dt.bfloat16, tag="rowtpv")
                    nc.tensor.transpose(ptv[:], statinv[:, i:i + 1], ident[:])
                    nc.vector.tensor_copy(rows_locv[:, 128 * i:128 * (i + 1)], ptv[:])

                # all-gather the stat rows (tiny, latency-bound)
                st_in = pdram.tile([3, TS], dt.bfloat16)
                st_out = pdram.tile([N_CORES, 3, TS], dt.bfloat16)
                nc.sync.dma_start(st_in[0:2, :], rows_loc[:])
                nc.sync.dma_start(st_in[2:3, :], rows_locv[:])
                nc.gpsimd.collective_compute(
                    "AllGather", alu.bypass,
                    replica_groups=[list(range(N_CORES))],
                    ins=[st_in.opt()],
                    outs=[st_out.opt()],
                )
                for r in range(N_CORES):
                    nc.sync.dma_start(rows_all[:, TS * r:TS * (r + 1)], st_out[r, 0:2, :])
                    nc.sync.dma_start(inv_row[:, TS * r:TS * (r + 1)], st_out[r, 2:3, :])

                # inv broadcast down partitions (evict on scalar engine)
                for ch in range(NCH):
                    pb = pps_a1.tile([128, 512], dt.float32, tag="invb")
                    nc.tensor.matmul(pb[:], ones128_row[:],
                                     inv_row[0:1, 512 * ch:512 * (ch + 1)],
                                     start=True, stop=True)
                    nc.scalar.copy(inv_b[:, 512 * ch:512 * (ch + 1)], pb[:])

                # x^T resident for the QKV matmuls, DMA'd per token-chunk
                xT = pxt.tile([128, NKC, TT], dt.bfloat16)
                for ch in range(NCH):
                    nc.sync.dma_start(
                        xT[:, :, 512 * ch:512 * (ch + 1)], p_xT[ch])

                vT = pxt.tile([128, TT], dt.bfloat16)
                for ch in range(NCH):
                    sl = slice(512 * ch, 512 * (ch + 1))
                    for (nm, w, cw, dst) in (("q", wq, cq, qT), ("k", wk, ck, kT),
                                             ("v", wv, cv, vT)):
                        ps = pps_a.tile([128, 512], dt.float32,
                                        name=f"ps{nm}", tag="qkv")
                        for k in range(NKC):
                            nc.tensor.matmul(ps[:], w[:, k, :], xT[:, k, sl],
                                             start=(k == 0), stop=False)
                        nc.tensor.matmul(ps[:], cw[:], rows_all[0:2, sl],
                                         start=False, stop=True)
                        nc.vector.tensor_tensor(dst[:, sl], ps[:], inv_b[:, sl],
                                                alu.mult)

                # v_aug [s, tile, head, 65] via PE transpose of vT; col 64 = 1
                nc.vector.memset(v[:, :, :, 64], 1.0)
                for i in range(NT):
                    pvt = pps_a1.tile([128, 128], dt.bfloat16, tag="vtp")
                    nc.tensor.transpose(pvt[:], vT[:, 128 * i:128 * (i + 1)],
                                        ident[:])
                    nc.scalar.copy(v[:, i, :, 0:64],
                                   pvt[:].rearrange("p (h d) -> p h d", h=2))

            # ---------------- stage B: attention ----------------
            with (
                tc.tile_pool(name="exps", bufs=6) as pexp,
                tc.tile_pool(name="attsb", bufs=2) as pat,
                tc.tile_pool(name="scpsum", bufs=3, space="PSUM") as pps_sc,
                tc.tile_pool(name="ctxpsum", bufs=2, space="PSUM") as pps_ctx,
                tc.tile_pool(name="zbpsum", bufs=1, space="PSUM") as pps_zb,
            ):
                # deferred normalization of the previous chunk: PE-side ops
                # (broadcast mms) are emitted in the middle of the next
                # chunk's score stream so the PE never stalls on the Z chain.
                def norm_pe_part(st):
                    pcp, zinvbp, gslp, cidxp = st
                    pzb = pps_zb.tile([128, 512], dt.float32, tag="zb")
                    for h in range(2):
                        nc.tensor.matmul(pzb[64 * h:64 * (h + 1), :],
                                         ones128_row[0:1, 0:64],
                                         zinvbp[h][:],
                                         start=True, stop=True)
                    zb = pat.tile([128, 512], dt.bfloat16, tag="zbs")
                    nc.vector.tensor_copy(zb[:], pzb[:])
                    for h in range(2):
                        nc.vector.tensor_tensor(
                            ctxT[64 * h:64 * (h + 1), gslp],
                            pcp[h][0:64, :], zb[64 * h:64 * (h + 1), :],
                            alu.mult)
                    nc.sync.dma_start(cc_in[cidxp], ctxT[:, gslp])

                pending = None
                for b in range(B):
                    for qt in range(T // 512):
                        G = b * T + 512 * qt
                        gsl = slice(G, G + 512)
                        cidx = b * (T // 512) + qt
                        nj = 4 * qt + 4
                        pc = [pps_ctx.tile([65, 512], dt.float32,
                                           name=f"pc{h}", tag=f"ctx{h}")
                              for h in range(2)]
                        ets = []
                        for j in range(nj):
                            st = b * (T // 128) + j   # global s-tile index
                            et2 = []
                            for h in range(2):
                                hsl = slice(64 * h, 64 * (h + 1))
                                ps = pps_sc.tile([128, 512], dt.float32,
                                                 name=f"ps{h}", tag="sc")
                                nc.tensor.matmul(
                                    ps[:], kT[hsl, 128 * st:128 * (st + 1)],
                                    qT[hsl, gsl], start=True, stop=True)
                                et = pexp.tile([128, 512], dt.bfloat16,
                                               name=f"et{h}", tag=f"et{h}")
                                if j >= nj - 4:
                                    off = j - (nj - 4)
                                    if off > 0:
                                        nc.gpsimd.memset(et[:, 0:128 * off], 0.0)
                                    nc.scalar.activation(
                                        et[:, 128 * off:512], ps[:, 128 * off:512],
                                        act.Exp, scale=1.0 / float(np.sqrt(H)))
                                    nc.gpsimd.tensor_tensor(
                                        et[:, 128 * off:128 * (off + 1)],
                                        et[:, 128 * off:128 * (off + 1)],
                                        maskd[:], alu.mult)
                                else:
                                    nc.scalar.activation(et[:], ps[:], act.Exp,
                                                         scale=1.0 / float(np.sqrt(H)))
                                et2.append(et)
                            ets.append(et2)
                            if j == 2 and pending is not None:
                                norm_pe_part(pending)
                                pending = None
                            # software pipeline: AV for tile j-1 after scores of j
                            if j > 0:
                                for h in range(2):
                                    nc.tensor.matmul(
                                        pc[h][:], v[:, b * (T // 128) + j - 1, h, :],
                                        ets[j - 1][h][:],
                                        start=(j - 1 == 0), stop=False)
                        for h in range(2):
                            nc.tensor.matmul(
                                pc[h][:], v[:, b * (T // 128) + nj - 1, h, :],
                                ets[nj - 1][h][:],
                                start=(nj == 1), stop=True)
                        # 1/Z (row 64 of each ctx psum) on the vector engine
                        zinvb = []
                        for h in range(2):
                            zrow = pat.tile([1, 512], dt.float32,
                                            name=f"zrow{h}", tag=f"z{h}")
                            nc.vector.tensor_copy(zrow[:], pc[h][64:65, :])
                            zinv = pat.tile([1, 512], dt.float32,
                                            name=f"zinv{h}", tag=f"zi{h}")
                            nc.vector.reciprocal_approx_fast(zinv[:], zrow[:])
                            zib = pat.tile([1, 512], dt.bfloat16,
                                           name=f"zinvb{h}", tag=f"zib{h}")
                            nc.vector.tensor_copy(zib[:], zinv[:])
                            zinvb.append(zib)
                        pending = (pc, zinvb, gsl, cidx)
                norm_pe_part(pending)
                pending = None

            # ---------------- AllToAll: heads -> tokens ----------------
            nc.gpsimd.collective_compute(
                "AllToAll", alu.bypass,
                replica_groups=[list(range(N_CORES))],
                ins=[cc_in.opt()],
                outs=[cc_out.opt()],
            )

            # ---------------- stage C: Wo + LN2 + FFN ----------------
            with (
                tc.tile_pool(name="postsb", bufs=1) as pq,
                tc.tile_pool(name="wstream", bufs=3) as pw,
                tc.tile_pool(name="evict", bufs=3) as pev,
                tc.tile_pool(name="ln2tmp", bufs=1) as pl2,
                tc.tile_pool(name="ffpsum", bufs=2, space="PSUM") as pps_ff,
                tc.tile_pool(name="cpsum", bufs=1, space="PSUM") as pps_c,
            ):
                ctxF = pq.tile([128, NKC, TS], dt.bfloat16)
                for j in range(N_CORES):
                    nc.sync.dma_start(ctxF[:, j, :], cc_out[j])

                bo = pq.tile([1, C], dt.bfloat16)
                nc.sync.dma_start(bo[:], p_bo[:])
                b2 = pq.tile([1, C], dt.bfloat16)
                nc.sync.dma_start(b2[:], p_b2[:])
                b1c = pq.tile([128, NMF], dt.float32)
                nc.sync.dma_start(b1c[:], p_b1c[:])
                xTs = pq.tile([128, NKC, TS], dt.float32)
                nc.sync.dma_start(xTs[:], p_xTs[:])

                # Wo + residual; LN2 column sums interleaved per block
                r2T = pq.tile([128, NKC, TS], dt.float32)
                r2b = pl2.tile([128, NKC, TS], dt.bfloat16)
                sq = pl2.tile([128, NKC, TS], dt.bfloat16)
                ps1 = pps_c.tile([1, TS], dt.float32, tag="s1")
                ps2 = pps_c.tile([1, TS], dt.float32, tag="s2")
                for mc in range(NKC):
                    wo_blk = pw.tile([128, NKC, 128], dt.bfloat16, tag="wo")
                    nc.sync.dma_start(wo_blk[:], p_woblk[mc])
                    ps = pps_ff.tile([128, TS], dt.float32, tag="ff")
                    for k in range(NKC):
                        nc.tensor.matmul(ps[:], wo_blk[:, k, :], ctxF[:, k, :],
                                         start=(k == 0), stop=False)
                    nc.tensor.matmul(ps[:], bo[0:1, 128 * mc:128 * (mc + 1)],
                                     ones_row[:], start=False, stop=True)
                    nc.vector.tensor_tensor(r2T[:, mc, :], ps[:], xTs[:, mc, :],
                                            alu.add)
                    nc.scalar.copy(r2b[:, mc, :], r2T[:, mc, :])
                    nc.vector.tensor_tensor(sq[:, mc, :], r2b[:, mc, :],
                                            r2b[:, mc, :], alu.mult)
                    nc.tensor.matmul(ps1[:], isc_col[:], r2b[:, mc, :],
                                     start=(mc == 0), stop=(mc == NKC - 1))
                    nc.tensor.matmul(ps2[:], isc_col[:], sq[:, mc, :],
                                     start=(mc == 0), stop=(mc == NKC - 1))

                # ---- LN2 scalars: mu, inv = rsqrt(var * C/(C-1)) ----
                muf = pl2.tile([1, TS], dt.float32)
                nc.vector.tensor_copy(muf[:], ps1[:])
                varf = pl2.tile([1, TS], dt.float32)
                nc.vector.tensor_tensor(varf[:], muf[:], muf[:], alu.mult)
                nc.vector.tensor_tensor(varf[:], ps2[:], varf[:], alu.subtract)
                stdf2 = pl2.tile([1, TS], dt.float32)
                nc.scalar.activation(stdf2[:], varf[:], act.Sqrt,
                                     scale=float(C) / (C - 1))
                nc.vector.tensor_scalar(stdf2[:], stdf2[:], EPS, None, alu.add)
                inv2 = pl2.tile([1, TS], dt.float32)
                nc.vector.reciprocal_approx_fast(inv2[:], stdf2[:])
                mu2row = pl2.tile([1, TS], dt.bfloat16)
                nc.vector.tensor_copy(mu2row[:], muf[:])
                inv2row = pl2.tile([1, TS], dt.bfloat16)
                nc.vector.tensor_copy(inv2row[:], inv2[:])
                pmb = pps_c.tile([128, TS], dt.float32, tag="bcast")
                nc.tensor.matmul(pmb[:], ones128_row[:], mu2row[:],
                                 start=True, stop=True)
                m2b = pl2.tile([128, TS], dt.bfloat16)
                nc.scalar.copy(m2b[:], pmb[:])
                pib = pps_c.tile([128, TS], dt.float32, tag="bcast")
                nc.tensor.matmul(pib[:], ones128_row[:], inv2row[:],
                                 start=True, stop=True)
                i2b = pl2.tile([128, TS], dt.bfloat16)
                nc.scalar.copy(i2b[:], pib[:])

                xn2T = pq.tile([128, NKC, TS], dt.bfloat16)
                for mc in range(NKC):
                    tmp = pev.tile([128, TS], dt.bfloat16, tag="xtmp")
                    nc.vector.tensor_tensor(tmp[:], r2T[:, mc, :], m2b[:],
                                            alu.subtract)
                    nc.vector.tensor_tensor(xn2T[:, mc, :], tmp[:], i2b[:],
                                            alu.mult)

                # ---- FFN ----
                hT = pq.tile([128, NMF, TS], dt.bfloat16)
                for mf in range(NMF):
                    w1_blk = pw.tile([128, NKC, 128], dt.bfloat16, tag="w1")
                    nc.sync.dma_start(w1_blk[:], p_w1blk[mf])
                    ps = pps_ff.tile([128, TS], dt.float32, tag="ff")
                    for k in range(NKC):
                        nc.tensor.matmul(ps[:], w1_blk[:, k, :], xn2T[:, k, :],
                                         start=(k == 0), stop=(k == NKC - 1))
                    nc.vector.tensor_scalar(hT[:, mf, :], ps[:], b1c[:, mf:mf + 1],
                                            0.0, alu.add, alu.max)

                for mc in range(NKC):
                    w2_blk = pw.tile([128, NMF, 128], dt.bfloat16, tag="w2")
                    nc.sync.dma_start(w2_blk[:], p_w2blk[mc])
                    ps = pps_ff.tile([128, TS], dt.float32, tag="ff")
                    for k in range(NMF):
                        nc.tensor.matmul(ps[:], w2_blk[:, k, :], hT[:, k, :],
                                         start=(k == 0), stop=False)
                    nc.tensor.matmul(ps[:], b2[0:1, 128 * mc:128 * (mc + 1)],
                                     ones_row[:], start=False, stop=True)
                    ot = pev.tile([128, TS], dt.float32, tag="ot")
                    nc.vector.tensor_tensor(ot[:], ps[:], r2T[:, mc, :], alu.add)
                    nc.sync.dma_start(p_out[128 * mc:128 * (mc + 1), :], ot[:])

    nc.compile()
    return nc


def _host_prep(inputs):
    """Fold layernorm affine params into weights; build per-core input maps."""
    x = np.asarray(inputs["x"], np.float32)
    Wq = np.asarray(inputs["Wq"], np.float32)
    Wk = np.asarray(inputs["Wk"], np.float32)
    Wv = np.asarray(inputs["Wv"], np.float32)
    Wo = np.asarray(inputs["Wo"], np.float32)
    bo = np.asarray(inputs["bo"], np.float32)
    W1 = np.asarray(inputs["W1"], np.float32)
    b1 = np.asarray(inputs["b1"], np.float32)
    W2 = np.asarray(inputs["W2"], np.float32)
    b2 = np.asarray(inputs["b2"], np.float32)
    g1 = np.asarray(inputs["g1"], np.float32)
    be1 = np.asarray(inputs["be1"], np.float32)
    g2 = np.asarray(inputs["g2"], np.float32)
    be2 = np.asarray(inputs["be2"], np.float32)

    xf = x.reshape(TT, C)                      # both batches stacked
    xT = np.ascontiguousarray(xf.T)            # [C, TT]
    # chunk-major, partition-contiguous: [NCH, 128, NKC, 512]
    xT_blk = np.ascontiguousarray(
        xT.reshape(NKC, 128, NCH, 512).transpose(2, 1, 0, 3))

    def fold_qkv(W):
        Weff = g1[:, None] * W                  # [NH, C, H] with g1 on C
        Weff = np.ascontiguousarray(np.transpose(Weff, (1, 0, 2)))  # [C, NH, H]
        bias = np.einsum("c,hck->hk", be1, W)   # [NH, H]
        colsum = Weff.sum(axis=0)               # [NH, H]
        return Weff, bias, colsum

    Wq_e, bq, csq = fold_qkv(Wq)
    Wk_e, bk, csk = fold_qkv(Wk)
    Wv_e, bv, csv = fold_qkv(Wv)

    woT = np.ascontiguousarray(Wo.T)            # [NH*H, C]
    w1T = np.ascontiguousarray(g2[:, None] * W1.T)   # [C, FF]
    b1_eff = b1 + be2 @ W1.T                         # [FF]
    w2T = np.ascontiguousarray(W2.T)            # [FF, C]

    # blocked weights [mblk, 128, K/128, 128], contiguous per partition
    woblk = np.ascontiguousarray(
        woT.reshape(NKC, 128, NKC, 128).transpose(2, 1, 0, 3))
    w1blk = np.ascontiguousarray(
        w1T.reshape(NKC, 128, NMF, 128).transpose(2, 1, 0, 3))
    w2blk = np.ascontiguousarray(
        w2T.reshape(NMF, 128, NKC, 128).transpose(2, 1, 0, 3))

    tq = np.arange(128)[None, :]
    s = np.arange(128)[:, None]
    maskd = (s <= tq).astype(BF16)

    x_bf = xf.astype(BF16)
    shared = {
        "p_xT": xT_blk.astype(BF16),
        "p_woblk": woblk.astype(BF16),
        "p_bo": bo[None, :].astype(BF16),
        "p_w1blk": w1blk.astype(BF16),
        "p_b1c": np.ascontiguousarray(
            b1_eff.reshape(NMF, 128).T).astype(np.float32),
        "p_w2blk": w2blk.astype(BF16),
        "p_b2": b2[None, :].astype(BF16),
        "p_maskd": maskd,
        "p_ident": np.eye(128, dtype=np.float32).astype(BF16),
    }

    in_maps = []
    for r in range(N_CORES):
        h0 = HPC * r
        hs = slice(h0, h0 + HPC)
        b_r, s_r = divmod(r, N_CORES // B)
        tok = slice(s_r * TS, (s_r + 1) * TS)
        xTs = np.ascontiguousarray(
            x[b_r].T[:, tok].reshape(NKC, 128, TS).transpose(1, 0, 2))
        m = dict(shared)
        m["p_x"] = x_bf[r * (TT // N_CORES):(r + 1) * (TT // N_CORES), :]
        m["p_xTs"] = xTs.astype(np.float32)
        m["p_wq"] = np.ascontiguousarray(
            Wq_e[:, hs, :].reshape(NKC, 128, HD2).transpose(1, 0, 2)).astype(BF16)
        m["p_wk"] = np.ascontiguousarray(
            Wk_e[:, hs, :].reshape(NKC, 128, HD2).transpose(1, 0, 2)).astype(BF16)
        m["p_wv"] = np.ascontiguousarray(
            Wv_e[:, hs, :].reshape(NKC, 128, HD2).transpose(1, 0, 2)).astype(BF16)
        m["p_cq"] = np.stack([csq[hs].reshape(HD2),
                              bq[hs].reshape(HD2)]).astype(BF16)
        m["p_ck"] = np.stack([csk[hs].reshape(HD2),
                              bk[hs].reshape(HD2)]).astype(BF16)
        m["p_cv"] = np.stack([csv[hs].reshape(HD2),
                              bv[hs].reshape(HD2)]).astype(BF16)
        in_maps.append(m)
    return in_maps


def kernel(**inputs) -> np.ndarray:
    from concourse.bass_utils import run_bass_kernel_spmd

    if "nc" not in _BUILT:
        _BUILT["nc"] = _build()
    nc = _BUILT["nc"]

    in_maps = _host_prep(inputs)
    res = run_bass_kernel_spmd(nc, in_maps, core_ids=list(range(N_CORES)))

    out = np.empty((B, T, C), np.float32)
    for r in range(N_CORES):
        b_r, s_r = divmod(r, N_CORES // B)
        out[b_r, s_r * TS:(s_r + 1) * TS, :] = res.results[r]["p_out"].T
    return out
